# revision 35
# baseline (speedup 1.0000x reference)
"""Multi-head attention (B=2, D=2048, N=1024, H=16) on 8 TRN2 NeuronCores.

Sharding: batch*heads across cores — core c handles batch c//4, heads
4*(c%4) .. 4*(c%4)+3. No collectives.

Fully-overlapped schedule (230us -> ~194us on HW):
  - Inputs are host-prearranged into the exact SBUF layouts so every DMA
    is a plain 2D copy (sub-us descriptor generation; DMA issue cost is
    set by destination contiguous-run length), shipped in ~0.5-1MB pieces
    in priority order (the DMA engines round-robin among in-flight
    transfers, so the critical first window must not queue behind bulk).
  - All projection operands are f16: f32r matmuls measured ~1.6 cyc/col
    (fp32-HIGH mode) vs 1.0 for f16, and f16 halves the input DMA bytes.
  - Attention for head 0 starts as soon as q(h0/h1, queries 0:1024) and
    k(keys 0:512) are projected (~24us in). Remaining projection chains
    interleave into the attention loop's PE slack one matmul at a time
    (the loop is paced by ScalarE's exp, 1147ns per [128,1024] tile);
    scores for iteration t+1 are emitted before PV of iteration t across
    head boundaries, so neither PE nor ScalarE drains at transitions.
  - All matmul stationaries are 128 rows x 128 cols: k tiles live in
    zero-padded per-head regions (kPad — the other parity's rows are
    zero, so contracting the full 128 partitions against the stacked
    q pair adds exact zeros), and v_ext tiles are [v | 1 | 0...] 128
    cols. Uniform row-groups let the PE pull every LDWEIGHTS into the
    background weight buffer behind the running matmul (mixed 64/128-row
    stationaries lost ~90ns per transition), and 128-col 16-bit weights
    engage the fast weight load path. All stationary regions are
    zero-initialized — uninitialized SBUF fed to the PE (even in
    never-read output rows) slowed every matmul ~20% (denormals).
  - PSUM: scores tiles [128,1024] x2 (1-ahead exp pipeline), one PV
    accumulator [128,1024], projection tiles [128,512] x2 = 8 banks.

Per-core math: qT/kT projection in transposed layout [head_dim, seq];
bias as per-partition scalar add on DVE. v in natural layout [seq, 64].
PV accumulates out_ext^T = v_ext^T expS^T; row 64 is the softmax
denominator. No softmax max-subtraction (|S|max ~ 52, exp fits fp32).
Host post-pass divides by the denominator, adds the (linearly separable)
v bias, reshapes to the reference's raw (B,H,D,p)->(B,D,N) layout.

dtypes: f16 projection and q/k, bf16 expS/v_ext.

Structural floor for this decomposition is ~170-175us: ScalarE exp is
128 x 1147ns = 147us (1 elem/cycle/lane, no DVE/GpSimd alternative — DVE
has no exp and its shift ALUs return 0, killing bit-trick exp), PE
streaming is ~150us (proj 41 + scores 54.6 + PV 54.6; the PE emits at
most 128 output elements per cycle, so the 16.7M score elements/core
need >= 131072 columns), plus ~10us fixed preamble/teardown. Things that
did NOT work: fp8/DoubleRow (e4m3 weight error >> 2e-2 gate), PV with
expS stationary (65-col moving makes it LDWEIGHTS-bound), splitting the
first block into half-width passes (longer pipeline fill ate the earlier
start), pre-emitting v chains before the first scores (delayed the
critical path).
"""
import sys

sys.path.insert(0, "/opt/trn_rl_repo")

import numpy as np
import ml_dtypes
import concourse.bacc as bacc
import concourse.mybir as mybir
from concourse import tile
from concourse.bass_utils import run_bass_kernel_spmd

B, D, N, H, P = 2, 2048, 1024, 16, 64
NCORES = 8
HPC = 4            # heads per core
KT = 8             # contraction tiles (N / 128)
ST = 4             # seq tiles of 512 for qk projection
JT = 16            # j (key) tiles of 128 per head
F32R = mybir.dt.float32r
F32 = mybir.dt.float32
BF16 = mybir.dt.bfloat16
F16 = mybir.dt.float16
EXP = mybir.ActivationFunctionType.Exp

PJ_DT = F16        # projection operands (x, W): f16 streams 1 col/cycle
                   # (f32r measured ~1.6 cyc/col in fp32-HIGH mode) and
                   # halves the input DMA bytes
QK_DT = F16        # q/k tiles feeding the scores matmul
PV_DT = BF16       # expS + v_ext feeding the PV matmul

# (head, ih) processing order: finish the h0/h1 pair (which only needs
# the m=0/m=2 projection blocks) before h2/h3 (m=1/m=3), so the second
# pair's projections can be interleaved into the first pair's attention.
ATTN_ORDER = [(0, 0), (1, 0), (0, 1), (1, 1),
              (2, 0), (3, 0), (2, 1), (3, 1)]
ITERS = [(h, ih, j) for (h, ih) in ATTN_ORDER for j in range(JT)]

_nc = None


class ChainRunner:
    """Drives projection chains (generators yielding per engine-op) with
    at most one partially-emitted chain at a time, so a later chain's
    first matmul can never deadlock the PE queue against an earlier
    chain's unemitted tail."""

    def __init__(self, make, disc_order):
        self.make = make          # key -> fresh generator
        self.done = set()
        self.cur_key = None
        self.cur_gen = None
        self.disc = list(disc_order)

    def _finish_current(self):
        if self.cur_gen is not None:
            for _ in self.cur_gen:
                pass
            self.done.add(self.cur_key)
            self.cur_key = self.cur_gen = None

    def ensure(self, key):
        if key in self.done:
            return
        if self.cur_key == key:
            self._finish_current()
            return
        self._finish_current()
        for _ in self.make(key):
            pass
        self.done.add(key)

    def drive(self, units):
        while units > 0:
            if self.cur_gen is None:
                while self.disc and self.disc[0] in self.done:
                    self.disc.pop(0)
                if not self.disc:
                    return
                self.cur_key = self.disc.pop(0)
                self.cur_gen = self.make(self.cur_key)
            try:
                next(self.cur_gen)
                units -= 1
            except StopIteration:
                self.done.add(self.cur_key)
                self.cur_key = self.cur_gen = None

    def finish_all(self):
        self._finish_current()
        while self.disc:
            key = self.disc.pop(0)
            if key not in self.done:
                self.ensure(key)


def _build():
    global _nc
    if _nc is not None:
        return _nc
    nc = bacc.Bacc("TRN2", target_bir_lowering=False, debug=False,
                   num_devices=NCORES)
    # All large inputs are host-prearranged into the exact SBUF layouts,
    # so every DMA is a plain 2D copy with >=8KB contiguous runs on both
    # sides (sub-us descriptor generation on the sync queue). xt is
    # shipped in ~1MB pieces so completion tracks issue order — the DMA
    # engines round-robin among in-flight transfers, so a single big
    # transfer issued first can still finish last.
    xt = nc.dram_tensor("xt", [128, ST * KT * 512], PJ_DT,
                        kind="ExternalInput").ap()
    # qk weights split by m-pair: wqk02 (q01|k01) is on the first
    # attention block's critical path, wqk13 (q23|k23) is not.
    wqk02 = nc.dram_tensor("wqk02", [128, KT * 256], PJ_DT,
                           kind="ExternalInput").ap()
    wqk13 = nc.dram_tensor("wqk13", [128, KT * 256], PJ_DT,
                           kind="ExternalInput").ap()
    wv = nc.dram_tensor("wv", [128, KT * 256], PJ_DT,
                        kind="ExternalInput").ap()
    bqk = nc.dram_tensor("bqk", [128, 4], F32, kind="ExternalInput").ap()
    o = nc.dram_tensor("o", [HPC, P + 1, D], F32, kind="ExternalOutput").ap()

    with tile.TileContext(nc) as tc:
        with tc.tile_pool(name="big", bufs=1) as big, \
             tc.tile_pool(name="es", bufs=6) as es, \
             tc.tile_pool(name="obp", bufs=2) as obp:

            # x^T staged s-major: [128, s(4) x k(8) x 512] — a per-s DMA
            # then writes one contiguous 16KB/partition destination block
            # (128 descriptors, ~0.7us issue on the sync queue, vs ~5us
            # for a 2KB-run destination).
            xt_t = big.tile([128, ST * KT * 512], PJ_DT, tag="xt")
            wqk02_t = big.tile([128, KT * 256], PJ_DT, tag="wqk02")
            wqk13_t = big.tile([128, KT * 256], PJ_DT, tag="wqk13")
            wv_t = big.tile([128, KT * 256], PJ_DT, tag="wv")
            bqk_t = big.tile([128, 4], F32, tag="bqk")
            # q in transposed pair layout [128 = 64 even | 64 odd, seq]
            qT = big.tile([128, 2 * D], QK_DT, tag="qT")
            # k in zero-padded per-head regions: head h occupies rows
            # bp..bp+64 of kPad[:, h*D : (h+1)*D]; the other rows are 0.
            kPad = big.tile([128, HPC * D], QK_DT, tag="kPad")
            # v_ext per (j, h): [v(64) | 1 | 0(63)] -> 128-col stationary
            vx = big.tile([128, JT * HPC * 128], PV_DT, tag="vx")

            # DMA order = arrival priority: qk weights, x^T windows s0/s1
            # (the first attention block's gate), wv, then s2/s3.
            nc.sync.dma_start(out=bqk_t[:], in_=bqk)
            nc.sync.dma_start(out=wqk02_t[:], in_=wqk02)
            for piece in (0, 1, 2, 3):  # s0, s1 — the first block's gate
                nc.sync.dma_start(
                    out=xt_t[:, piece * 2048:(piece + 1) * 2048],
                    in_=xt[:, piece * 2048:(piece + 1) * 2048])
            nc.sync.dma_start(out=wv_t[:], in_=wv)
            for piece in (4, 5, 6, 7):  # s2, s3
                nc.sync.dma_start(
                    out=xt_t[:, piece * 2048:(piece + 1) * 2048],
                    in_=xt[:, piece * 2048:(piece + 1) * 2048])
            # wqk13 last: its consumers (q/k projections for heads 2/3)
            # are discretionary fillers not needed until mid-stream.
            nc.sync.dma_start(out=wqk13_t[:], in_=wqk13)

            # zero-init the padded k regions and v_ext (garbage bits in
            # PE weight cells can be denormals, which slow the MAC
            # array); the ones column of v_ext is col 64 of each block.
            # kPad first — its consumer (the k-projection evac) runs
            # ~6us before the first v evac.
            nc.gpsimd.memset(kPad[:], 0.0)
            nc.gpsimd.memset(vx[:], 0.0)
            nc.gpsimd.memset(
                vx.rearrange("p (t c) -> p t c", c=128)[:, :, 64:65], 1.0)

            with tc.tile_pool(name="ps", bufs=2, space="PSUM") as ps, \
                 tc.tile_pool(name="po", bufs=1, space="PSUM") as po, \
                 tc.tile_pool(name="pj", bufs=2, space="PSUM") as pj:

                def qk_chain(m, s):
                    pt = pj.tile([128, 512], F32, tag="pj",
                                 name=f"pq{m}{s}")
                    wt = wqk02_t if m in (0, 2) else wqk13_t
                    wo = 128 if m >= 2 else 0
                    for k in range(KT):
                        nc.tensor.matmul(
                            pt[:],
                            wt[:, k * 256 + wo:k * 256 + wo + 128],
                            xt_t[:, s * 4096 + k * 512:
                                 s * 4096 + (k + 1) * 512],
                            start=(k == 0), stop=(k == KT - 1))
                        yield
                    if m < 2:
                        # q pair block: rows 0:64 = even head, 64:128 = odd
                        nc.vector.tensor_scalar_add(
                            qT[:, m * D + s * 512:m * D + (s + 1) * 512],
                            pt[:], bqk_t[:, m:m + 1])
                        yield
                    else:
                        # k block: scatter the two heads into their
                        # zero-padded regions (even head rows 0:64,
                        # odd head rows 64:128).
                        he = 2 * (m - 2)
                        nc.vector.tensor_scalar_add(
                            kPad[0:64, he * D + s * 512:
                                 he * D + (s + 1) * 512],
                            pt[0:64, :], bqk_t[0:64, m:m + 1])
                        yield
                        nc.vector.tensor_scalar_add(
                            kPad[64:128, (he + 1) * D + s * 512:
                                 (he + 1) * D + (s + 1) * 512],
                            pt[64:128, :], bqk_t[64:128, m:m + 1])
                        yield

                def v_chain(j, pr):
                    # one head-pair's v for key-tile j: block 0 (heads
                    # 0/1) then only forces the pair-0 chains; pair-1
                    # drains through the discretionary queue before the
                    # h2/h3 blocks need it.
                    pt = pj.tile([128, 128], F32, tag="pj",
                                 name=f"pv{j}{pr}")
                    xo = (j // 4) * 4096 + (j % 4) * 128
                    for k in range(KT):
                        nc.tensor.matmul(
                            pt[:],
                            xt_t[:, xo + k * 512:xo + k * 512 + 128],
                            wv_t[:, k * 256 + pr * 128:
                                 k * 256 + (pr + 1) * 128],
                            start=(k == 0), stop=(k == KT - 1))
                        yield
                    for hh in (2 * pr, 2 * pr + 1):
                        nc.vector.tensor_copy(
                            vx[:, (j * HPC + hh) * 128:
                               (j * HPC + hh) * 128 + 64],
                            pt[:, (hh - 2 * pr) * 64:
                               (hh - 2 * pr + 1) * 64])
                    yield

                def make(key):
                    if key[0] == "qk":
                        return qk_chain(key[1], key[2])
                    return v_chain(key[1], key[2])

                # discretionary pre-drive order: k(s0) before q(s1) so
                # the k chain runs inside the s1 DMA wait window; the
                # first exp's gate is then just the q(s1) chain.
                pre = [("qk", 0, 0), ("qk", 2, 0), ("qk", 0, 1)]
                # qk chains drain through disc ahead of the pair-1 v
                # chains: an ensure-force of a 9-matmul qk chain lands as
                # one ~2us lump on the exp stream, while v chains force
                # fine-grained (one short chain per iteration).
                seen = set(pre)
                disc = []
                for vpass in (False, True):
                    for (h, ih, j) in ITERS:
                        mq, mk = (0, 2) if h < 2 else (1, 3)
                        keys = ([("v", j, h // 2)] if vpass else
                                [("qk", mk, j // 4), ("qk", mq, 2 * ih),
                                 ("qk", mq, 2 * ih + 1)])
                        for key in keys:
                            if key not in seen:
                                disc.append(key)
                                seen.add(key)
                runner = ChainRunner(make, disc)
                for key in pre:
                    runner.ensure(key)

                def emit_scores(h, ih, j):
                    st = ps.tile([128, 1024], F32, tag="ps", name="st")
                    qoff = (h // 2) * D + ih * 1024
                    for i2 in range(2):
                        nc.tensor.matmul(
                            st[:, i2 * 512:(i2 + 1) * 512],
                            kPad[:, h * D + j * 128:h * D + (j + 1) * 128],
                            qT[:, qoff + i2 * 512:qoff + (i2 + 1) * 512],
                            start=True, stop=True)
                    return st

                sts = {0: emit_scores(*ITERS[0])}
                ot = None
                for t, (h, ih, j) in enumerate(ITERS):
                    if t + 1 < len(ITERS):
                        hn, ihn, jn = ITERS[t + 1]
                        mq, mk = (0, 2) if hn < 2 else (1, 3)
                        runner.ensure(("qk", mq, 2 * ihn))
                        runner.ensure(("qk", mq, 2 * ihn + 1))
                        runner.ensure(("qk", mk, jn // 4))
                        sts[t + 1] = emit_scores(hn, ihn, jn)
                    et = es.tile([128, 1024], PV_DT, tag="et", name="et")
                    nc.scalar.activation(et[:], sts.pop(t)[:], EXP)
                    runner.ensure(("v", j, h // 2))
                    if t >= JT:
                        runner.drive(1)
                    if j == 0:
                        ot = po.tile([128, 1024], F32, tag="po", name="ot")
                    for i2 in range(2):
                        nc.tensor.matmul(
                            ot[:, i2 * 512:(i2 + 1) * 512],
                            vx[:, (j * HPC + h) * 128:
                               (j * HPC + h + 1) * 128],
                            et[:, i2 * 512:(i2 + 1) * 512],
                            start=(j == 0), stop=(j == JT - 1))
                    if j == JT - 1:
                        # per-half evacuation: half 0's accumulation
                        # closed with the i2=0 matmul above, so its copy
                        # overlaps the i2=1 matmul stream.
                        ob = obp.tile([P + 1, 1024], F32, tag="ob",
                                      name="ob")
                        od = o.rearrange("h p d -> (h p) d")[
                            h * 65:(h + 1) * 65,
                            ih * 1024:(ih + 1) * 1024]
                        for i2 in range(2):
                            nc.vector.tensor_copy(
                                ob[:, i2 * 512:(i2 + 1) * 512],
                                ot[0:P + 1, i2 * 512:(i2 + 1) * 512])
                            nc.sync.dma_start(
                                out=od[:, i2 * 512:(i2 + 1) * 512],
                                in_=ob[:, i2 * 512:(i2 + 1) * 512])
                runner.finish_all()
    nc.compile()
    _nc = nc
    return nc


def _np_dt(dt):
    if dt == BF16:
        return ml_dtypes.bfloat16
    if dt == mybir.dt.float16:
        return np.float16
    return np.float32


def _shard_inputs(x, W_qkv, b_qkv):
    pj = _np_dt(PJ_DT)
    in_maps = []
    for c in range(NCORES):
        b = c // 4
        h0 = HPC * (c % 4)
        # x^T staged in the SBUF layout [p, s(4), k(8), 512]:
        # element (p, s, k, d) = x[b][s*512 + d, k*128 + p]
        xT = x[b].T.reshape(KT, 128, ST, 512).transpose(1, 2, 0, 3)
        xT = np.ascontiguousarray(xT.reshape(128, ST * KT * 512)).astype(pj)
        wq = W_qkv[:, h0 * P:(h0 + HPC) * P]
        wk = W_qkv[:, N + h0 * P:N + (h0 + HPC) * P]
        # qk weights in SBUF layout [p, k(8), 256], split by m-pair:
        # wqk02 = [q heads 0,1 | k heads 0,1], wqk13 = [q 2,3 | k 2,3]
        wqk = np.concatenate([wq, wk], axis=1).reshape(KT, 128, 4, 128)
        w02 = wqk[:, :, (0, 2)].reshape(KT, 128, 256)
        w13 = wqk[:, :, (1, 3)].reshape(KT, 128, 256)
        wqk02 = np.ascontiguousarray(
            w02.transpose(1, 0, 2).reshape(128, KT * 256)).astype(pj)
        wqk13 = np.ascontiguousarray(
            w13.transpose(1, 0, 2).reshape(128, KT * 256)).astype(pj)
        wv = W_qkv[:, 2 * N + h0 * P:2 * N + (h0 + HPC) * P]
        wv = wv.reshape(KT, 128, 256)
        wv = np.ascontiguousarray(
            wv.transpose(1, 0, 2).reshape(128, KT * 256)).astype(pj)
        bq = b_qkv[h0 * P:(h0 + HPC) * P]
        bk = b_qkv[N + h0 * P:N + (h0 + HPC) * P]
        bqk = np.ascontiguousarray(
            np.concatenate([bq, bk]).reshape(4, 128).T).astype(np.float32)
        in_maps.append({"xt": xT, "wqk02": wqk02, "wqk13": wqk13,
                        "wv": wv, "bqk": bqk})
    return in_maps


def _assemble(results, b_qkv):
    out = np.empty((B, D, N), dtype=np.float32)
    for c in range(NCORES):
        b = c // 4
        h0 = HPC * (c % 4)
        oe = results[c]["o"]                      # (4, 65, 2048)
        att = oe[:, :P, :] / oe[:, P:P + 1, :]    # (4, 64, 2048)
        att = np.transpose(att, (0, 2, 1))        # (4, 2048, 64)
        for hl in range(HPC):
            h = h0 + hl
            bv = b_qkv[2 * N + h * P:2 * N + (h + 1) * P]
            out[b, h * 128:(h + 1) * 128, :] = \
                (att[hl] + bv[None, :]).reshape(128, N)
    return out


def _forward(in_maps, **kwargs):
    nc = _build()
    return run_bass_kernel_spmd(nc, in_maps, core_ids=list(range(NCORES)),
                                **kwargs)


def kernel(x, W_qkv, b_qkv):
    x = np.asarray(x, dtype=np.float32)
    W_qkv = np.asarray(W_qkv, dtype=np.float32)
    b_qkv = np.asarray(b_qkv, dtype=np.float32)
    in_maps = _shard_inputs(x, W_qkv, b_qkv)
    res = _forward(in_maps)
    return _assemble(res.results, b_qkv)
